# revision 1
# baseline (speedup 1.0000x reference)
"""Holt-Winters exponential smoothing (level/trend/seasonal, P=7) on 8 Trainium2
NeuronCores — v11: v6 with bf16-hi-only group state (no lo plane).

Math identical to v3 (C=105 chunks, 13/group, 3 groups, channel-planar
pass-2, bf16 weights/x/out, fp32 PSUM, sigma_hi-only pass-2, bf16-hi-only
group state; validated relL2 4.8e-3 against float64, gate 2e-2).
Schedule changes vs v3:
  - all weights + s0 coalesced into ONE dram tensor / ONE dma (v3 lost
    35 us to a 13-dma weight-load stall before the first matmul);
  - input group 0 split across the sync and gpsimd queues so the first
    scan starts ~14 us in;
  - PSUM retiled to one-bank tiles: scan 2 x (126,512), pass-2 6 x
    (105,512) -- a 6-deep matmul->cast pipeline that decouples PE pacing
    from cast turnaround (HAM cold-clock trap);
  - output casts alternate DVE / ACT per tile.
"""

import numpy as np

P = 7
C = 105
G = 13
NG = 3
NCH = G * NG
KS = 114          # pass-2 rhs rows: 105 X + 9 sigma_hi
L = 4096
B = 8192
NCORES = 8
BL = B // NCORES
NH = 512

# wall (weights+s0) column offsets
WP0 = 0           # (114, 315)
WQ0 = 315         # (105, 1638)
WS0 = WP0 + 315 + 1638          # ws1 (9, 126)
S00 = WS0 + 126                 # s0 (9, 2048)
WALLW = S00 + 2 * BL            # 4127


def _sigmoid(z):
    return 1.0 / (1.0 + np.exp(-z))


def _step_mats(a, b, g):
    A, c = [], []
    for i in range(P):
        col = 2 + i
        Ai = np.zeros((9, 9), np.float64)
        ci = np.zeros(9, np.float64)
        Ai[0, 0] = 1 - a
        Ai[0, 1] = 1 - a
        Ai[0, col] += -a
        Ai[1, 0] = -a * b
        Ai[1, 1] = 1 - a * b
        Ai[1, col] += -a * b
        for j in range(P):
            Ai[2 + j, 2 + j] = 1.0
        Ai[col, :] = 0.0
        Ai[col, 0] = -g * (1 - a)
        Ai[col, 1] = -g * (1 - a)
        Ai[col, col] = g * a + 1 - g
        ci[0] = a
        ci[1] = a * b
        ci[col] = g * (1 - a)
        A.append(Ai)
        c.append(ci)
    return A, c


def _build_coeffs(alpha, beta, gamma):
    """Weight blocks in float64; packed into the per-core wall later."""
    a, b, g = _sigmoid(alpha), _sigmoid(beta), _sigmoid(gamma)
    A, c = _step_mats(a, b, g)
    slots = [(1 + k) % P for k in range(C)]

    Phi = np.zeros((C, 9, 9), np.float64)
    w = np.zeros((C, C, 9), np.float64)
    cur = np.eye(9)
    for k in range(C):
        i = slots[k]
        if k > 0:
            w[k, :k] = w[k - 1, :k] @ A[i].T
        w[k, k] = c[i]
        cur = A[i] @ cur
        Phi[k] = cur
    T = Phi[C - 1]
    V = w[C - 1].T.copy()

    wp = np.zeros((KS, 3 * C), np.float64)          # [ch0|ch1|ch2]
    for k in range(C):
        sel = [0, 1, 2 + slots[k]]
        for ch in range(3):
            wp[105:114, ch * C + k] = Phi[k][sel[ch]]
            for j in range(k + 1):
                wp[j, ch * C + k] = w[k, j][sel[ch]]

    Tpow = [np.eye(9)]
    for _ in range(G + 1):
        Tpow.append(T @ Tpow[-1])

    ws1 = np.zeros((9, 126), np.float64)
    ws1[:, 0:9] = Tpow[G].T
    for j in range(G):
        ws1[:, 9 + 9 * j:18 + 9 * j] = Tpow[j].T
    wq = np.zeros((C, G * 126), np.float64)         # [i0|i1|...|i12]
    for i in range(G):
        blk = wq[:, i * 126:(i + 1) * 126]
        blk[:, 0:9] = (Tpow[G - 1 - i] @ V).T
        for j in range(i + 1, G):
            blk[:, 9 + 9 * j:18 + 9 * j] = (Tpow[j - 1 - i] @ V).T

    return wp, wq, ws1


def build_bass(bl=BL):
    import concourse.bacc as bacc
    import concourse.mybir as mybir
    from concourse.tile import TileContext

    BF = mybir.dt.bfloat16
    F32 = mybir.dt.float32
    COPY = mybir.ActivationFunctionType.Copy
    nh = bl // NH
    GW = G * bl

    nc = bacc.Bacc(None, target_bir_lowering=False, debug=False)
    xin = nc.declare_dram_parameter("xin", [C, NCH * bl], BF, isOutput=False)
    wall_d = nc.declare_dram_parameter("wall", [KS, WALLW], BF,
                                       isOutput=False)
    out_d = nc.declare_dram_parameter("out", [C, NCH * 3 * bl], BF,
                                      isOutput=True)

    with TileContext(nc) as tc:
        with (
            tc.tile_pool(name="consts", bufs=1) as consts,
            tc.tile_pool(name="xpool", bufs=NG) as xpool,
            tc.tile_pool(name="spool", bufs=2) as spool,
            tc.tile_pool(name="ypool", bufs=4) as ypool,
            tc.tile_pool(name="ypsum", bufs=6, space="PSUM") as ypsum,
            tc.tile_pool(name="spsum", bufs=2, space="PSUM") as spsum,
        ):
            cw = consts.tile([KS, WALLW], BF)
            # s0 block first (tiny, gates the first scan matmuls), then the
            # weight block; rows 9:114 of the s0 region are never read, so
            # skip them instead of DMAing 430KB of zero padding.
            nc.sync.dma_start(out=cw[0:9, S00:WALLW], in_=wall_d[0:9, S00:WALLW])
            nc.sync.dma_start(out=cw[:, 0:S00], in_=wall_d[:, 0:S00])
            wp = cw[:, WP0:WP0 + 3 * C]
            wq = cw[0:C, WQ0:WQ0 + G * 126]
            ws1 = cw[0:9, WS0:WS0 + 126]
            s0 = cw[0:9, S00:S00 + 2 * bl]

            # fine-grained input loads: 2-chunk pieces for g0 (fast start,
            # 4KB partition lines), 3-4-chunk pieces for g1/g2; spread over
            # the sync and gpsimd queues to engage all engines early.
            xg = []
            pieces = {0: ([(0, 2, 1), (2, 4, 1), (4, 6, 1), (6, 8, 1),
                           (8, 10, 1), (10, 12, 1), (12, 13, 1)]),
                      1: ([(0, 4, 0), (4, 8, 0), (8, 11, 0), (11, 13, 0)]),
                      2: ([(0, 4, 1), (4, 8, 1), (8, 11, 1), (11, 13, 1)])}
            for g_ in range(NG):
                xt = xpool.tile([KS, GW], BF, tag="xg", name=f"xg{g_}")
                for (a, b, q) in pieces[g_]:
                    src = xin[:, g_ * GW + a * bl:g_ * GW + b * bl]
                    dst = xt[0:C, a * bl:b * bl]
                    if q == 0:
                        nc.sync.dma_start(out=dst, in_=src)
                    else:
                        nc.gpsimd.dma_start(out=dst, in_=src)
                xg.append(xt)

            state = [s0[:, 0:bl]]

            def scan_split_scatter(g_):
                st_h = state[g_]
                sps = []
                for h in range(nh):
                    hs = slice(h * NH, (h + 1) * NH)
                    sp = spsum.tile([126, NH], F32, tag="sp",
                                    name=f"sp{g_}_{h}")
                    nc.tensor.matmul(sp[:], lhsT=ws1[:], rhs=st_h[:, hs],
                                     start=True, stop=False)
                    for i in range(G):
                        nc.tensor.matmul(sp[:],
                                         lhsT=wq[:, i * 126:(i + 1) * 126],
                                         rhs=xg[g_][0:C, i * bl + h * NH:
                                                    i * bl + h * NH + NH],
                                         start=False, stop=(i == G - 1))
                    sps.append(sp)
                sg = spool.tile([126, bl], BF, tag="sg", name=f"sg{g_}")
                for h in range(nh):
                    hs = slice(h * NH, (h + 1) * NH)
                    nc.scalar.activation(out=sg[:, hs], in_=sps[h][:],
                                         func=COPY)
                for i in range(G):
                    nc.sync.dma_start(
                        out=xg[g_][105:114, i * bl:(i + 1) * bl],
                        in_=sg[9 + 9 * i:18 + 9 * i, :])
                state.append(sg[0:9, :])

            def pass2_chunk(g_, i):
                ot = ypool.tile([C, 3 * bl], BF, tag="ot", name=f"ot{g_}_{i}")
                k = 0
                for h in range(nh):
                    chs = (0, 1, 2) if h == 0 else (2, 1, 0)
                    for ch in chs:
                        yp = ypsum.tile([C, NH], F32, tag="yp",
                                        name=f"yp{g_}_{i}_{h}_{ch}")
                        nc.tensor.matmul(
                            yp[:],
                            lhsT=wp[:, ch * C:(ch + 1) * C],
                            rhs=xg[g_][0:KS, i * bl + h * NH:
                                       i * bl + h * NH + NH],
                            start=True, stop=True)
                        oc = slice((h * 3 + ch) * NH, (h * 3 + ch + 1) * NH)
                        if k % 2 == 0:
                            nc.vector.tensor_copy(out=ot[:, oc], in_=yp[:])
                        else:
                            nc.scalar.activation(out=ot[:, oc], in_=yp[:],
                                                 func=COPY)
                        k += 1
                c0 = (g_ * G + i) * 3 * bl
                if g_ == NG - 1 and i % 2 == 1:
                    nc.sync.dma_start(out=out_d[:, c0:c0 + 3 * bl], in_=ot[:])
                else:
                    nc.gpsimd.dma_start(out=out_d[:, c0:c0 + 3 * bl],
                                        in_=ot[:])

            scan_split_scatter(0)
            for g_ in range(NG):
                for i in range(G):
                    if i == 3 and g_ + 1 < NG:
                        scan_split_scatter(g_ + 1)
                    pass2_chunk(g_, i)
    nc.compile()
    return nc


def _prep_inputs(x, alpha, beta, gamma):
    import ml_dtypes
    bf = ml_dtypes.bfloat16
    xs = np.asarray(x, dtype=np.float32).reshape(B, L)
    wp, wq, ws1 = _build_coeffs(float(alpha), float(beta), float(gamma))
    wall0 = np.zeros((KS, WALLW), np.float32)
    wall0[:, WP0:WP0 + 3 * C] = wp
    wall0[0:C, WQ0:WQ0 + G * 126] = wq
    wall0[0:9, WS0:WS0 + 126] = ws1
    in_maps = []
    for m in range(NCORES):
        xm = xs[m * BL:(m + 1) * BL]
        xT = np.ascontiguousarray(xm.T)
        xb = xT.astype(bf)
        xin = np.ascontiguousarray(
            xb[1:L].reshape(NCH, C, BL).transpose(1, 0, 2)).reshape(
                C, NCH * BL)
        s0 = np.zeros((9, BL), np.float32)
        s0[0] = xT[0]
        s0[1] = xT[1] - xT[0]
        for j in range(1, P):
            s0[2 + j] = xT[j] - xT[0]
        s0h = s0.astype(bf)
        s0l = (s0 - s0h.astype(np.float32)).astype(bf)
        wall = wall0.copy()
        wall[0:9, S00:S00 + BL] = s0h
        wall[0:9, S00 + BL:S00 + 2 * BL] = s0l
        in_maps.append({"xin": xin, "wall": wall.astype(bf)})
    return in_maps


LAST_RESULT = None

def _ensure_ntff_hook():
    """If BASS_TRACE is set but this environment lacks antenv.axon_hooks
    (concourse imports it under axon when tracing), provide it -- registered
    from the injected libaxon_pjrt.so when available, else a no-op so
    run_bass_kernel_spmd degrades to an untraced run instead of crashing."""
    import importlib.util
    try:
        if importlib.util.find_spec("antenv.axon_hooks") is not None:
            return
    except (ImportError, ModuleNotFoundError, ValueError):
        pass
    import contextlib
    import ctypes
    import sys
    import types

    mod = types.ModuleType("antenv.axon_hooks")
    mod._hook = None
    mod.set_axon_ntff_profile_hook = lambda h: setattr(mod, "_hook", h)
    mod.get_axon_ntff_profile_hook = lambda: mod._hook
    sys.modules["antenv.axon_hooks"] = mod
    try:
        import antenv
        antenv.axon_hooks = mod
    except ImportError:
        pass
    try:
        lib = ctypes.CDLL("/opt/axon/libaxon_pjrt.so")
        if not hasattr(lib, "axon_start_nrt_profile"):
            return
        lib.axon_start_nrt_profile.argtypes = [
            ctypes.POINTER(ctypes.c_int64), ctypes.c_size_t]
        lib.axon_start_nrt_profile.restype = ctypes.c_int64
        lib.axon_stop_nrt_profile.argtypes = [ctypes.c_char_p]
        lib.axon_stop_nrt_profile.restype = ctypes.c_int64

        @contextlib.contextmanager
        def _hook(output_dir, device_ids):
            import jax
            jax.devices()
            if device_ids:
                ids = (ctypes.c_int64 * len(device_ids))(*device_ids)
                rc = lib.axon_start_nrt_profile(ids, len(device_ids))
            else:
                rc = lib.axon_start_nrt_profile(None, 0)
            if rc != 0:
                raise RuntimeError(f"axon_start_nrt_profile rc={rc}")
            try:
                yield
            finally:
                lib.axon_stop_nrt_profile(str(output_dir).encode())

        mod.set_axon_ntff_profile_hook(_hook)
    except OSError:
        pass



def kernel(x, alpha, beta, gamma):
    global LAST_RESULT
    _ensure_ntff_hook()
    from concourse.bass_utils import run_bass_kernel_spmd

    nc = build_bass(BL)
    in_maps = _prep_inputs(x, alpha, beta, gamma)
    res = run_bass_kernel_spmd(nc, in_maps, core_ids=list(range(NCORES)))
    LAST_RESULT = res
    xs = np.asarray(x, dtype=np.float32).reshape(B, L)
    y = np.empty((B, L, 3), np.float32)
    y[:, 0, 0] = xs[:, 0]
    y[:, 0, 1] = xs[:, 1] - xs[:, 0]
    y[:, 0, 2] = 0.0
    for m in range(NCORES):
        o = res.results[m]["out"]
        o = o.reshape(C, NCH, 2, 3, NH).astype(np.float32)
        y[m * BL:(m + 1) * BL, 1:, :] = o.transpose(2, 4, 1, 0, 3).reshape(
            BL, L - 1, 3)
    return y

